# revision 31
# baseline (speedup 1.0000x reference)
"""Trainium2 Bass kernel for BiLinearSigmoidAttention (v3).

Reference math (per batch b, with L = length[b]):
    qn = l2norm(query), cn = l2norm(context)
    raw[q,k] = qn[q] . cn[k]            (masked: k >= L -> -1e30)
    sig = sigmoid(raw)
    den[q] = max(sum_k sig[q,k], 1)
    scores[q,k] = sig[q,k] / den[q]     (rows q >= L zeroed)
    att[q,:] = sum_k scores[q,k] * context[k,:]
    out = concat([qn, att], -1)
returns (out [B,S,2D], scores [B,S,S])

v3 design (8 NeuronCores, data parallel over B=32 -> 4 slots per core):
  - All compute and IO in bf16 (fp32 PSUM accumulation).
  - Length sparsity: only the top-left ceil(L/128)-block square of the
    score matrix is nonzero. Batches are sorted by length and dealt
    round-robin to cores so all 8 cores run ONE identical program whose
    per-slot block count is the max over that slot's 8 batches. The
    host zero-fills the rest of scores/att.
  - mm1 computes scoresT [k_part, q_free]; the key mask is a
    per-partition bias and the context l2-norm a per-partition scale,
    fused into the ACT sigmoid eviction.
  - Software-pipelined emission: slot i+1's loads/norms run on DVE/DMA
    while the PE does slot i's mm1, and slot i+1's transposes issue
    right after slot i's mm2, so every engine queue stays busy.
  - inv-norms via fast-inverse-sqrt (int seed + 1 Newton step) on DVE:
    the ACT activation table stays on Sigmoid (table reloads are 1.3us).
  - GPSIMD only gets tiny [P,1] ops (its elementwise rate is ~16x worse
    than DVE); eviction work is split ACT/DVE.
  - Warm-up matmuls from a memset tile release the PE HAM clock gate
    before the first real matmuls arrive.
"""

import numpy as np

import concourse.bacc as bacc
import concourse.mybir as mybir
import concourse.tile as tile
from concourse.bass_utils import run_bass_kernel_spmd

try:
    import ml_dtypes

    BF16 = np.dtype(ml_dtypes.bfloat16)
except ImportError:  # pragma: no cover
    BF16 = None

B, S, D = 32, 1024, 512
NCORES = 8
NSLOTS = B // NCORES       # 4 slots (batches) per core
P = 128                    # partitions
NT = S // P                # 8 s-tiles
ND = D // P                # 4 d-chunks
NEG = np.float32(-1e30)

F32 = mybir.dt.float32
I32 = mybir.dt.int32
BF = mybir.dt.bfloat16
F8 = mybir.dt.float8e4
AF = mybir.ActivationFunctionType
ALU = mybir.AluOpType

N_WARM = 16  # PE warm-up matmuls (N=512) at kernel start


def build_kernel(slot_lbs):
    """slot_lbs: tuple of NSLOTS ints, block count (ceil(L/128)) per slot."""
    nc = bacc.Bacc("TRN2", target_bir_lowering=False, debug=False)

    qs, cs, kbs, qms, qns, atts, scs = [], [], [], [], [], [], []
    for i, lb in enumerate(slot_lbs):
        nb = lb * P
        qs.append(nc.dram_tensor(f"q{i}", [S, D], BF, kind="ExternalInput"))
        cs.append(nc.dram_tensor(f"c{i}", [nb, D], BF, kind="ExternalInput"))
        # kb[p, kt] = 0 if kt*P+p < L else -1e30 ; qm[p, qt] = 1/0
        kbs.append(nc.dram_tensor(f"kb{i}", [P, lb], F32, kind="ExternalInput"))
        qms.append(nc.dram_tensor(f"qm{i}", [P, lb], F32, kind="ExternalInput"))
        qns.append(nc.dram_tensor(f"qn{i}", [S, D], BF, kind="ExternalOutput"))
        atts.append(nc.dram_tensor(f"att{i}", [nb, D], BF, kind="ExternalOutput"))
        scs.append(nc.dram_tensor(f"sc{i}", [nb, nb], BF, kind="ExternalOutput"))
    id_d = nc.dram_tensor("ident", [P, P], BF, kind="ExternalInput")
    on_d = nc.dram_tensor("ones", [P, 2], BF, kind="ExternalInput")

    with tile.TileContext(nc) as tc:
        _body(tc, slot_lbs, qs, cs, kbs, qms, qns, atts, scs, id_d, on_d)
    nc.compile()
    return nc


def _body(tc, slot_lbs, qs, cs, kbs, qms, qns, atts, scs, id_d, on_d):
    nc = tc.nc
    from contextlib import ExitStack

    ctx = ExitStack()
    with ctx:
        const = ctx.enter_context(tc.tile_pool(name="const", bufs=1))
        qpool = ctx.enter_context(tc.tile_pool(name="q", bufs=2))
        cpool = ctx.enter_context(tc.tile_pool(name="c", bufs=2))
        tpool = ctx.enter_context(tc.tile_pool(name="t", bufs=2))
        sgpool = ctx.enter_context(tc.tile_pool(name="sg", bufs=2))
        mpool = ctx.enter_context(tc.tile_pool(name="m", bufs=2))
        spool = ctx.enter_context(tc.tile_pool(name="s", bufs=2))
        opool = ctx.enter_context(tc.tile_pool(name="o", bufs=2))
        wpool = ctx.enter_context(tc.tile_pool(name="w", bufs=2))
        # PSUM budget is 8 banks: acc/den 2 + pt 3 + att 2 + dcol 1
        ps1 = ctx.enter_context(tc.tile_pool(name="ps1", bufs=2, space="PSUM"))
        pst = ctx.enter_context(tc.tile_pool(name="pst", bufs=3, space="PSUM"))
        ps2 = ctx.enter_context(tc.tile_pool(name="ps2", bufs=2, space="PSUM"))
        psd = ctx.enter_context(tc.tile_pool(name="psd", bufs=1, space="PSUM"))

        ident = const.tile([P, P], BF, tag="ident")
        ones = const.tile([P, 2], BF, tag="ones")
        warm = const.tile([P, 512], BF, tag="warm")
        # Warm-up operands come from a memset (no DMA dependency): the PE
        # starts within ~1us of kernel entry and releases the HAM clock gate
        # while the input DMAs stream in.
        nc.vector.memset(warm[:], 1.0)
        for _ in range(N_WARM):
            wp = ps2.tile([P, 512], F32, tag="att")
            nc.tensor.matmul(wp[:], warm[:, 0:P], warm[:], start=True, stop=True)
        nc.sync.dma_start(ident[:], id_d[:])
        nc.sync.dma_start(ones[:], on_d[:])

        st = {}

        def rsqrt(inv_ap, ssq_ap, t1, t2, final_scale=None):
            """inv = ssq**-0.5 on DVE: fast-inverse-sqrt seed + 1 Newton."""
            nc.vector.tensor_scalar(
                t1.bitcast(I32)[:], ssq_ap.bitcast(I32)[:], 1, None,
                op0=ALU.logical_shift_right,
            )
            nc.vector.tensor_scalar(
                t2.bitcast(I32)[:], t1.bitcast(I32)[:], -1, None,
                op0=ALU.bitwise_xor,
            )
            nc.vector.tensor_scalar(
                t1.bitcast(I32)[:], t2.bitcast(I32)[:], 0x5F3759E0,
                None, op0=ALU.add,
            )
            nc.vector.scalar_tensor_tensor(
                t2[:], t1[:], 1.0, t1[:], op0=ALU.mult, op1=ALU.mult
            )
            nc.vector.tensor_tensor(t2[:], t2[:], ssq_ap[:], op=ALU.mult)
            nc.vector.tensor_scalar(
                t2[:], t2[:], -0.5, 1.5, op0=ALU.mult, op1=ALU.add
            )
            if final_scale is None:
                nc.vector.tensor_tensor(inv_ap[:], t1[:], t2[:], op=ALU.mult)
            else:
                nc.vector.scalar_tensor_tensor(
                    inv_ap[:], t1[:], final_scale, t2[:],
                    op0=ALU.mult, op1=ALU.mult,
                )

        def front_pre(i):
            """Loads + norms + qn scaling + qn output: DMA/DVE(/ACT) only.
            Emitted before slot i-1's mm2 so it runs during mm1(i-1).
            The q-chain (squares -> rsqrt -> scale) gates the transposes, so
            it is fully independent of the c-chain (separate ssq tiles and
            scratch rings; c-norms are only needed by mm1's sigmoids)."""
            lb = slot_lbs[i]
            qt = qpool.tile([P, NT, D], BF, tag="qt", name=f"qt{i}")
            qnf = qpool.tile([P, NT, D], BF, tag="qnf", name=f"qnf{i}")
            ct = cpool.tile([P, NT, D], BF, tag="ct", name=f"ct{i}")
            kb = mpool.tile([P, NT], F32, tag="kb", name=f"kb{i}")
            qm = mpool.tile([P, NT], F32, tag="qm", name=f"qm{i}")
            qre = qs[i].rearrange("(t p) d -> p t d", p=P)
            nc.sync.dma_start(
                ct[:, 0:lb], cs[i].rearrange("(t p) d -> p t d", p=P)
            )
            nc.sync.dma_start(qt[:, 0 : NT // 2], qre[:, 0 : NT // 2])
            nc.sync.dma_start(qt[:, NT // 2 :], qre[:, NT // 2 :])
            nc.sync.dma_start(kb[:, 0:lb], kbs[i][:])
            nc.sync.dma_start(qm[:, 0:lb], qms[i][:])

            ssq_q = mpool.tile([P, NT], F32, tag="ssq_q", name=f"ssqq{i}")
            ssq_c = mpool.tile([P, NT], F32, tag="ssq_c", name=f"ssqc{i}")
            inv_q = mpool.tile([P, NT], F32, tag="inv_q", name=f"invq{i}")
            inv_c = mpool.tile([P, NT], F32, tag="inv_c", name=f"invc{i}")
            # q-chain first: fused square+row-sum, rsqrt, scale into a fresh
            # tile (dense non-inplace copy gets the fast DVE mode). Slot 0
            # runs it as two half-chains so the first q-tiles transpose while
            # the second DMA half is still in flight.
            t1q = wpool.tile([P, NT], F32, tag="t1q", name=f"t1q{i}")
            t2q = wpool.tile([P, NT], F32, tag="t2q", name=f"t2q{i}")
            halves = ((0, NT // 2), (NT // 2, NT)) if i == 0 else ((0, NT),)
            for (h0, h1) in halves:
                for t in range(h0, h1):
                    scr = spool.tile([P, D], BF, tag="scrq", name=f"scrq{i}_{t}")
                    nc.vector.scalar_tensor_tensor(
                        scr[:], qt[:, t], 1.0, qt[:, t],
                        op0=ALU.mult, op1=ALU.mult,
                        accum_out=ssq_q[:, t : t + 1],
                    )
                rsqrt(
                    inv_q[:, h0:h1], ssq_q[:, h0:h1],
                    t1q[:, h0:h1], t2q[:, h0:h1],
                )
                for t in range(h0, h1):
                    nc.vector.tensor_scalar_mul(
                        qnf[:, t], qt[:, t], inv_q[:, t : t + 1]
                    )
            nc.gpsimd.dma_start(qns[i].rearrange("(t p) d -> p t d", p=P), qnf[:])

            # c-chain: slot 0 runs it on ACT (everything else is idle and the
            # result is only needed once mm1 evictions start)
            for t in range(lb):
                scr = spool.tile([P, D], BF, tag="scrc", name=f"scrc{i}_{t}")
                if i == 0:
                    nc.scalar.activation(
                        scr[:], ct[:, t], AF.Square,
                        accum_out=ssq_c[:, t : t + 1],
                    )
                else:
                    nc.vector.scalar_tensor_tensor(
                        scr[:], ct[:, t], 1.0, ct[:, t],
                        op0=ALU.mult, op1=ALU.mult,
                        accum_out=ssq_c[:, t : t + 1],
                    )
            t1c = wpool.tile([P, NT], F32, tag="t1c", name=f"t1c{i}")
            t2c = wpool.tile([P, NT], F32, tag="t2c", name=f"t2c{i}")
            rsqrt(
                inv_c[:, 0:lb], ssq_c[:, 0:lb], t1c[:, 0:lb], t2c[:, 0:lb],
                final_scale=1.0 / 16.0,
            )
            st[i] = dict(lb=lb, qnf=qnf, ct=ct, kb=kb, qm=qm, inv_c=inv_c)

        def front_T(i):
            """PE transposes into [d, s] layout; c first (it needs no norms),
            then normalized q. qT evicts on ACT, cT evicts on DVE."""
            s_ = st[i]
            lb, qnf, ct = s_["lb"], s_["qnf"], s_["ct"]
            qT = tpool.tile([P, ND, S], F8, tag="qT", name=f"qT{i}")
            cT = tpool.tile([P, ND, S], F8, tag="cT", name=f"cT{i}")
            t = 0
            while t < lb:
                n2 = min(2, lb - t)
                pc = pst.tile([P, ND, 2 * P], BF, tag="pt", name=f"pc{i}_{t}")
                for j in range(n2):
                    for dch in range(ND):
                        nc.tensor.transpose(
                            pc[:, dch, j * P : (j + 1) * P],
                            ct[:, t + j, dch * P : (dch + 1) * P], ident[:],
                        )
                eng = nc.scalar if (i == 0 and (t // 2) % 2 == 0) else None
                if eng is not None:
                    nc.scalar.activation(
                        cT[:, :, t * P : (t + n2) * P],
                        pc[:, :, 0 : n2 * P], AF.Copy, scale=2.0,
                    )
                else:
                    nc.vector.tensor_scalar_mul(
                        cT[:, :, t * P : (t + n2) * P],
                        pc[:, :, 0 : n2 * P], 2.0,
                    )
                t += n2
            if i == 0:
                for _ in range(10):
                    wp = ps2.tile([P, 512], F32, tag="att", name="warmb")
                    nc.tensor.matmul(
                        wp[:], warm[:, 0:P], warm[:], start=True, stop=True
                    )
            t = 0
            while t < lb:
                n2 = min(2, lb - t)
                pq = pst.tile([P, ND, 2 * P], BF, tag="pt", name=f"pq{i}_{t}")
                for j in range(n2):
                    for dch in range(ND):
                        nc.tensor.transpose(
                            pq[:, dch, j * P : (j + 1) * P],
                            qnf[:, t + j, dch * P : (dch + 1) * P], ident[:],
                        )
                nc.scalar.activation(
                    qT[:, :, t * P : (t + n2) * P],
                    pq[:, :, 0 : n2 * P], AF.Copy, scale=8.0,
                )
                t += n2
            s_["qT"], s_["cT"] = qT, cT

        def emit_mm1(i):
            """sigT[k, q] = sigmoid(cT.T @ qT * inv_c + keybias)."""
            s_ = st[i]
            lb, qT, cT, kb = s_["lb"], s_["qT"], s_["cT"], s_["kb"]
            inv_c = s_["inv_c"]
            nb = lb * P
            qchunks = [(j * 512, min(512, nb - j * 512))
                       for j in range((nb + 511) // 512)]
            sg = sgpool.tile([P, NT, S], BF, tag="sg", name=f"sg{i}")
            for ci, (q0, qn_) in enumerate(qchunks):
                for kt in range(lb):
                    acc = ps1.tile(
                        [P, 512], F32, tag="acc", name=f"acc{i}_{kt}_{ci}"
                    )
                    for dp in range(ND // 2):
                        nc.tensor.matmul(
                            acc[:, 0:qn_],
                            cT[:, 2 * dp : 2 * dp + 2, kt * P : (kt + 1) * P],
                            qT[:, 2 * dp : 2 * dp + 2, q0 : q0 + qn_],
                            start=(dp == 0),
                            stop=(dp == ND // 2 - 1),
                            perf_mode=mybir.MatmulPerfMode.DoubleRow,
                        )
                    nc.scalar.activation(
                        sg[:, kt, q0 : q0 + qn_], acc[:, 0:qn_],
                        AF.Sigmoid, bias=kb[:, kt : kt + 1],
                        scale=inv_c[:, kt : kt + 1],
                    )
            s_["sg"] = sg
            s_["qchunks"] = qchunks

        def emit_mm2(i):
            """den in row form (ones-column matmuls over sigT), transposed to
            per-partition columns on the PE; then per q-block att + sigT
            transposes, evicted with w = qmask * min(1/den, 1)."""
            s_ = st[i]
            lb, ct, qm, sg = s_["lb"], s_["ct"], s_["qm"], s_["sg"]
            qchunks = s_["qchunks"]
            nb = lb * P

            # den rows: den[0, q] = sum_k sigT[k, q]
            den_sb = mpool.tile([1, S], BF, tag="den", name=f"den{i}")
            for ci, (q0, qn_) in enumerate(qchunks):
                dp = ps1.tile([P, 512], F32, tag="acc", name=f"dp{i}_{ci}")
                for kt in range(lb):
                    nc.tensor.matmul(
                        dp[0:1, 0:qn_], ones[:, 0:1], sg[:, kt, q0 : q0 + qn_],
                        start=(kt == 0), stop=(kt == lb - 1),
                    )
                nc.scalar.copy(den_sb[0:1, q0 : q0 + qn_], dp[0:1, 0:qn_])
            # transpose den to columns: dcol[p, qb] = den[qb*P + p]
            dcol = psd.tile([P, NT, 2], BF, tag="dcol", name=f"dcol{i}")
            for qb in range(lb):
                nc.tensor.transpose(
                    dcol[:, qb, 0:1],
                    den_sb[0:1, qb * P : (qb + 1) * P],
                    ones[0:1, 0:1],
                )
            # w_all = qmask * min(1/den, 1)  (den > 0 always)
            winv = wpool.tile([P, NT], F32, tag="winv", name=f"wi{i}")
            w_all = wpool.tile([P, NT], F32, tag="w", name=f"w{i}")
            nc.vector.reciprocal(winv[:, 0:lb], dcol[:, 0:lb, 0])
            nc.vector.scalar_tensor_tensor(
                w_all[:, 0:lb], winv[:, 0:lb], 1.0, qm[:, 0:lb],
                op0=ALU.min, op1=ALU.mult,
            )

            ao_all = opool.tile([P, NT, D], BF, tag="ao", name=f"ao{i}")
            so_all = opool.tile([P, NT, S], BF, tag="so", name=f"so{i}")
            for qb in range(lb):
                att = ps2.tile([P, 512], F32, tag="att", name=f"att{i}_{qb}")
                pt = pst.tile([P, NT * P], BF, tag="pt", name=f"pt{i}_{qb}")
                for kt in range(lb):
                    sgblk = sg[:, kt, qb * P : (qb + 1) * P]
                    nc.tensor.matmul(
                        att[:], sgblk, ct[:, kt],
                        start=(kt == 0), stop=(kt == lb - 1),
                    )
                    nc.tensor.transpose(
                        pt[:, kt * P : (kt + 1) * P], sgblk, ident[:]
                    )
                w = w_all[:, qb : qb + 1]
                if qb % 2 == 0:
                    nc.vector.tensor_scalar_mul(ao_all[:, qb], att[:], w)
                else:
                    nc.scalar.activation(ao_all[:, qb], att[:], AF.Copy, scale=w)
                if qb % 2 == 0:
                    nc.scalar.activation(
                        so_all[:, qb, 0:nb], pt[:, 0:nb], AF.Copy, scale=w
                    )
                else:
                    nc.vector.tensor_scalar_mul(
                        so_all[:, qb, 0:nb], pt[:, 0:nb], w
                    )
            nc.gpsimd.dma_start(
                atts[i].rearrange("(t p) d -> p t d", p=P), ao_all[:, 0:lb]
            )
            screar = scs[i].rearrange("(t p) k -> p t k", p=P)
            if lb >= 4:
                h = lb // 2
                nc.gpsimd.dma_start(screar[:, 0:h], so_all[:, 0:h, 0:nb])
                nc.gpsimd.dma_start(screar[:, h:lb], so_all[:, h:lb, 0:nb])
            else:
                nc.gpsimd.dma_start(screar[:, 0:lb], so_all[:, 0:lb, 0:nb])

        front_pre(0)
        front_T(0)
        for i in range(NSLOTS):
            emit_mm1(i)
            if i + 1 < NSLOTS:
                front_pre(i + 1)
            emit_mm2(i)
            if i + 1 < NSLOTS:
                front_T(i + 1)


_NC_CACHE = {}


def _get_nc(slot_lbs):
    key = tuple(slot_lbs)
    if key not in _NC_CACHE:
        _NC_CACHE[key] = build_kernel(key)
    return _NC_CACHE[key]


def _plan(length):
    """Sort batches desc by length, deal rank r -> (slot r//8, core r%8)."""
    order = np.argsort(-length, kind="stable")
    slot_lbs = []
    for i in range(NSLOTS):
        lmax = int(length[order[i * NCORES]])
        for r in range(i * NCORES, (i + 1) * NCORES):
            lmax = max(lmax, int(length[order[r]]))
        slot_lbs.append(max(1, (lmax + P - 1) // P))
    return order, tuple(slot_lbs)


def kernel(context, query, length):
    context = np.asarray(context, dtype=np.float32)
    query = np.asarray(query, dtype=np.float32)
    length = np.asarray(length).astype(np.int64)

    order, slot_lbs = _plan(length)

    q_bf = query.astype(BF16)
    c_bf = context.astype(BF16)
    iot = np.arange(S)
    keymask = iot[None, :] < length[:, None]                      # [B, S]
    kbH = np.where(keymask, np.float32(0.0), NEG).astype(np.float32)
    kbH = np.ascontiguousarray(kbH.reshape(B, NT, P).transpose(0, 2, 1))
    qmH = keymask.astype(np.float32)
    qmH = np.ascontiguousarray(qmH.reshape(B, NT, P).transpose(0, 2, 1))

    in_maps = []
    for c in range(NCORES):
        m = {
            "ident": np.eye(P, dtype=np.float32).astype(BF16),
            "ones": np.ones((P, 2), dtype=np.float32).astype(BF16),
        }
        for i, lb in enumerate(slot_lbs):
            b = int(order[i * NCORES + c])
            nb = lb * P
            m[f"q{i}"] = np.ascontiguousarray(q_bf[b])
            m[f"c{i}"] = np.ascontiguousarray(c_bf[b, :nb])
            m[f"kb{i}"] = np.ascontiguousarray(kbH[b, :, :lb])
            m[f"qm{i}"] = np.ascontiguousarray(qmH[b, :, :lb])
        in_maps.append(m)

    nc = _get_nc(slot_lbs)
    res = run_bass_kernel_spmd(nc, in_maps, list(range(NCORES)))
    _NC_CACHE["last_result"] = res

    out = np.zeros((B, S, 2 * D), dtype=np.float32)
    scores = np.zeros((B, S, S), dtype=np.float32)
    for i, lb in enumerate(slot_lbs):
        nb = lb * P
        for c in range(NCORES):
            b = int(order[i * NCORES + c])
            r = res.results[c]
            out[b, :, 0:D] = r[f"qn{i}"].astype(np.float32)
            out[b, 0:nb, D : 2 * D] = r[f"att{i}"].astype(np.float32)
            scores[b, 0:nb, 0:nb] = r[f"sc{i}"].astype(np.float32)
    return out, scores


# revision 32
# speedup vs baseline: 1.2044x; 1.2044x over previous
"""Trainium2 Bass kernel for BiLinearSigmoidAttention (v3).

Reference math (per batch b, with L = length[b]):
    qn = l2norm(query), cn = l2norm(context)
    raw[q,k] = qn[q] . cn[k]            (masked: k >= L -> -1e30)
    sig = sigmoid(raw)
    den[q] = max(sum_k sig[q,k], 1)
    scores[q,k] = sig[q,k] / den[q]     (rows q >= L zeroed)
    att[q,:] = sum_k scores[q,k] * context[k,:]
    out = concat([qn, att], -1)
returns (out [B,S,2D], scores [B,S,S])

v3 design (8 NeuronCores, data parallel over B=32 -> 4 slots per core):
  - All compute and IO in bf16 (fp32 PSUM accumulation).
  - Length sparsity: only the top-left ceil(L/128)-block square of the
    score matrix is nonzero. Batches are sorted by length and dealt
    round-robin to cores so all 8 cores run ONE identical program whose
    per-slot block count is the max over that slot's 8 batches. The
    host zero-fills the rest of scores/att.
  - mm1 computes scoresT [k_part, q_free]; the key mask is a
    per-partition bias and the context l2-norm a per-partition scale,
    fused into the ACT sigmoid eviction.
  - Software-pipelined emission: slot i+1's loads/norms run on DVE/DMA
    while the PE does slot i's mm1, and slot i+1's transposes issue
    right after slot i's mm2, so every engine queue stays busy.
  - inv-norms via fast-inverse-sqrt (int seed + 1 Newton step) on DVE:
    the ACT activation table stays on Sigmoid (table reloads are 1.3us).
  - GPSIMD only gets tiny [P,1] ops (its elementwise rate is ~16x worse
    than DVE); eviction work is split ACT/DVE.
  - Warm-up matmuls from a memset tile release the PE HAM clock gate
    before the first real matmuls arrive.
"""

import numpy as np

import concourse.bacc as bacc
import concourse.mybir as mybir
import concourse.tile as tile
from concourse.bass_utils import run_bass_kernel_spmd

try:
    import ml_dtypes

    BF16 = np.dtype(ml_dtypes.bfloat16)
except ImportError:  # pragma: no cover
    BF16 = None

B, S, D = 32, 1024, 512
NCORES = 8
NSLOTS = B // NCORES       # 4 slots (batches) per core
P = 128                    # partitions
NT = S // P                # 8 s-tiles
ND = D // P                # 4 d-chunks
NEG = np.float32(-1e30)

F32 = mybir.dt.float32
I32 = mybir.dt.int32
BF = mybir.dt.bfloat16
F8 = mybir.dt.float8e4
AF = mybir.ActivationFunctionType
ALU = mybir.AluOpType

N_WARM = 16  # PE warm-up matmuls (N=512) at kernel start


def build_kernel(slot_lbs):
    """slot_lbs: tuple of NSLOTS ints, block count (ceil(L/128)) per slot."""
    nc = bacc.Bacc("TRN2", target_bir_lowering=False, debug=False)

    qs, cs, kbs, qms, qns, atts, scs = [], [], [], [], [], [], []
    for i, lb in enumerate(slot_lbs):
        nb = lb * P
        qs.append(nc.dram_tensor(f"q{i}", [S, D], BF, kind="ExternalInput"))
        cs.append(nc.dram_tensor(f"c{i}", [nb, D], BF, kind="ExternalInput"))
        # kb[p, kt] = 0 if kt*P+p < L else -1e30 ; qm[p, qt] = 1/0
        kbs.append(nc.dram_tensor(f"kb{i}", [P, lb], F32, kind="ExternalInput"))
        qms.append(nc.dram_tensor(f"qm{i}", [P, lb], F32, kind="ExternalInput"))
        qns.append(nc.dram_tensor(f"qn{i}", [S, D], BF, kind="ExternalOutput"))
        atts.append(nc.dram_tensor(f"att{i}", [nb, D], BF, kind="ExternalOutput"))
        scs.append(nc.dram_tensor(f"sc{i}", [nb, nb], BF, kind="ExternalOutput"))
    id_d = nc.dram_tensor("ident", [P, P], BF, kind="ExternalInput")
    on_d = nc.dram_tensor("ones", [P, 2], BF, kind="ExternalInput")

    with tile.TileContext(nc) as tc:
        _body(tc, slot_lbs, qs, cs, kbs, qms, qns, atts, scs, id_d, on_d)
    nc.compile()
    return nc


def _body(tc, slot_lbs, qs, cs, kbs, qms, qns, atts, scs, id_d, on_d):
    nc = tc.nc
    from contextlib import ExitStack

    ctx = ExitStack()
    with ctx:
        const = ctx.enter_context(tc.tile_pool(name="const", bufs=1))
        qpool = ctx.enter_context(tc.tile_pool(name="q", bufs=2))
        cpool = ctx.enter_context(tc.tile_pool(name="c", bufs=2))
        tpool = ctx.enter_context(tc.tile_pool(name="t", bufs=2))
        sgpool = ctx.enter_context(tc.tile_pool(name="sg", bufs=2))
        mpool = ctx.enter_context(tc.tile_pool(name="m", bufs=2))
        spool = ctx.enter_context(tc.tile_pool(name="s", bufs=2))
        opool = ctx.enter_context(tc.tile_pool(name="o", bufs=2))
        wpool = ctx.enter_context(tc.tile_pool(name="w", bufs=2))
        # PSUM budget is 8 banks: acc/den 2 + pt 3 + att 2 + dcol 1
        ps1 = ctx.enter_context(tc.tile_pool(name="ps1", bufs=2, space="PSUM"))
        pst = ctx.enter_context(tc.tile_pool(name="pst", bufs=3, space="PSUM"))
        ps2 = ctx.enter_context(tc.tile_pool(name="ps2", bufs=2, space="PSUM"))
        psd = ctx.enter_context(tc.tile_pool(name="psd", bufs=1, space="PSUM"))

        ident = const.tile([P, P], BF, tag="ident")
        ones = const.tile([P, 2], BF, tag="ones")
        warm = const.tile([P, 512], BF, tag="warm")
        # Warm-up operands come from a memset (no DMA dependency): the PE
        # starts within ~1us of kernel entry and releases the HAM clock gate
        # while the input DMAs stream in.
        nc.vector.memset(warm[:], 1.0)
        for _ in range(N_WARM):
            wp = ps2.tile([P, 512], F32, tag="att")
            nc.tensor.matmul(wp[:], warm[:, 0:P], warm[:], start=True, stop=True)
        nc.sync.dma_start(ident[:], id_d[:])
        nc.sync.dma_start(ones[:], on_d[:])

        st = {}

        def rsqrt(inv_ap, ssq_ap, t1, t2, final_scale=None):
            """inv = ssq**-0.5 on DVE: fast-inverse-sqrt seed + 1 Newton."""
            nc.vector.tensor_scalar(
                t1.bitcast(I32)[:], ssq_ap.bitcast(I32)[:], 1, None,
                op0=ALU.logical_shift_right,
            )
            nc.vector.tensor_scalar(
                t2.bitcast(I32)[:], t1.bitcast(I32)[:], -1, None,
                op0=ALU.bitwise_xor,
            )
            nc.vector.tensor_scalar(
                t1.bitcast(I32)[:], t2.bitcast(I32)[:], 0x5F3759E0,
                None, op0=ALU.add,
            )
            nc.vector.scalar_tensor_tensor(
                t2[:], t1[:], 1.0, t1[:], op0=ALU.mult, op1=ALU.mult
            )
            nc.vector.tensor_tensor(t2[:], t2[:], ssq_ap[:], op=ALU.mult)
            nc.vector.tensor_scalar(
                t2[:], t2[:], -0.5, 1.5, op0=ALU.mult, op1=ALU.add
            )
            if final_scale is None:
                nc.vector.tensor_tensor(inv_ap[:], t1[:], t2[:], op=ALU.mult)
            else:
                nc.vector.scalar_tensor_tensor(
                    inv_ap[:], t1[:], final_scale, t2[:],
                    op0=ALU.mult, op1=ALU.mult,
                )

        def front_pre(i):
            """Loads + norms + qn scaling + qn output: DMA/DVE(/ACT) only.
            Emitted before slot i-1's mm2 so it runs during mm1(i-1).
            The q-chain (squares -> rsqrt -> scale) gates the transposes, so
            it is fully independent of the c-chain (separate ssq tiles and
            scratch rings; c-norms are only needed by mm1's sigmoids)."""
            lb = slot_lbs[i]
            qt = qpool.tile([P, NT, D], BF, tag="qt", name=f"qt{i}")
            qnf = qpool.tile([P, NT, D], BF, tag="qnf", name=f"qnf{i}")
            ct = cpool.tile([P, NT, D], BF, tag="ct", name=f"ct{i}")
            kb = mpool.tile([P, NT], F32, tag="kb", name=f"kb{i}")
            qm = mpool.tile([P, NT], F32, tag="qm", name=f"qm{i}")
            qre = qs[i].rearrange("(t p) d -> p t d", p=P)
            nc.sync.dma_start(
                ct[:, 0:lb], cs[i].rearrange("(t p) d -> p t d", p=P)
            )
            nc.sync.dma_start(qt[:, 0 : NT // 2], qre[:, 0 : NT // 2])
            nc.sync.dma_start(qt[:, NT // 2 :], qre[:, NT // 2 :])
            nc.sync.dma_start(kb[:, 0:lb], kbs[i][:])
            nc.sync.dma_start(qm[:, 0:lb], qms[i][:])

            ssq_q = mpool.tile([P, NT], F32, tag="ssq_q", name=f"ssqq{i}")
            ssq_c = mpool.tile([P, NT], F32, tag="ssq_c", name=f"ssqc{i}")
            inv_q = mpool.tile([P, NT], F32, tag="inv_q", name=f"invq{i}")
            inv_c = mpool.tile([P, NT], F32, tag="inv_c", name=f"invc{i}")
            # q-chain first: fused square+row-sum, rsqrt, scale into a fresh
            # tile (dense non-inplace copy gets the fast DVE mode). Slot 0
            # runs it as two half-chains so the first q-tiles transpose while
            # the second DMA half is still in flight.
            t1q = wpool.tile([P, NT], F32, tag="t1q", name=f"t1q{i}")
            t2q = wpool.tile([P, NT], F32, tag="t2q", name=f"t2q{i}")
            halves = ((0, NT // 2), (NT // 2, NT)) if i == 0 else ((0, NT),)
            for (h0, h1) in halves:
                for t in range(h0, h1):
                    scr = spool.tile([P, D], BF, tag="scrq", name=f"scrq{i}_{t}")
                    nc.vector.scalar_tensor_tensor(
                        scr[:], qt[:, t], 1.0, qt[:, t],
                        op0=ALU.mult, op1=ALU.mult,
                        accum_out=ssq_q[:, t : t + 1],
                    )
                rsqrt(
                    inv_q[:, h0:h1], ssq_q[:, h0:h1],
                    t1q[:, h0:h1], t2q[:, h0:h1],
                )
                for t in range(h0, h1):
                    nc.vector.tensor_scalar_mul(
                        qnf[:, t], qt[:, t], inv_q[:, t : t + 1]
                    )
            nc.gpsimd.dma_start(qns[i].rearrange("(t p) d -> p t d", p=P), qnf[:])

            # c-chain: slot 0 runs it on ACT (everything else is idle and the
            # result is only needed once mm1 evictions start)
            for t in range(lb):
                scr = spool.tile([P, D], BF, tag="scrc", name=f"scrc{i}_{t}")
                if i == 0:
                    nc.scalar.activation(
                        scr[:], ct[:, t], AF.Square,
                        accum_out=ssq_c[:, t : t + 1],
                    )
                else:
                    nc.vector.scalar_tensor_tensor(
                        scr[:], ct[:, t], 1.0, ct[:, t],
                        op0=ALU.mult, op1=ALU.mult,
                        accum_out=ssq_c[:, t : t + 1],
                    )
            t1c = wpool.tile([P, NT], F32, tag="t1c", name=f"t1c{i}")
            t2c = wpool.tile([P, NT], F32, tag="t2c", name=f"t2c{i}")
            rsqrt(
                inv_c[:, 0:lb], ssq_c[:, 0:lb], t1c[:, 0:lb], t2c[:, 0:lb],
                final_scale=1.0 / 16.0,
            )
            st[i] = dict(lb=lb, qnf=qnf, ct=ct, kb=kb, qm=qm, inv_c=inv_c)

        def front_T(i):
            """PE transposes into [d, s] layout; c first (it needs no norms),
            then normalized q. qT evicts on ACT, cT evicts on DVE."""
            s_ = st[i]
            lb, qnf, ct = s_["lb"], s_["qnf"], s_["ct"]
            qT = tpool.tile([P, ND, S], F8, tag="qT", name=f"qT{i}")
            cT = tpool.tile([P, ND, S], F8, tag="cT", name=f"cT{i}")
            t = 0
            while t < lb:
                n2 = min(2, lb - t)
                pc = pst.tile([P, ND, 2 * P], BF, tag="pt", name=f"pc{i}_{t}")
                for j in range(n2):
                    for dch in range(ND):
                        nc.tensor.transpose(
                            pc[:, dch, j * P : (j + 1) * P],
                            ct[:, t + j, dch * P : (dch + 1) * P], ident[:],
                        )
                eng = nc.scalar if (i == 0 and (t // 2) % 2 == 0) else None
                if eng is not None:
                    nc.scalar.activation(
                        cT[:, :, t * P : (t + n2) * P],
                        pc[:, :, 0 : n2 * P], AF.Copy, scale=2.0,
                    )
                else:
                    nc.vector.tensor_scalar_mul(
                        cT[:, :, t * P : (t + n2) * P],
                        pc[:, :, 0 : n2 * P], 2.0,
                    )
                t += n2
            t = 0
            while t < lb:
                n2 = min(2, lb - t)
                pq = pst.tile([P, ND, 2 * P], BF, tag="pt", name=f"pq{i}_{t}")
                for j in range(n2):
                    for dch in range(ND):
                        nc.tensor.transpose(
                            pq[:, dch, j * P : (j + 1) * P],
                            qnf[:, t + j, dch * P : (dch + 1) * P], ident[:],
                        )
                nc.scalar.activation(
                    qT[:, :, t * P : (t + n2) * P],
                    pq[:, :, 0 : n2 * P], AF.Copy, scale=8.0,
                )
                t += n2
            s_["qT"], s_["cT"] = qT, cT

        def emit_mm1(i):
            """sigT[k, q] = sigmoid(cT.T @ qT * inv_c + keybias)."""
            s_ = st[i]
            lb, qT, cT, kb = s_["lb"], s_["qT"], s_["cT"], s_["kb"]
            inv_c = s_["inv_c"]
            nb = lb * P
            qchunks = [(j * 512, min(512, nb - j * 512))
                       for j in range((nb + 511) // 512)]
            sg = sgpool.tile([P, NT, S], BF, tag="sg", name=f"sg{i}")
            for ci, (q0, qn_) in enumerate(qchunks):
                for kt in range(lb):
                    acc = ps1.tile(
                        [P, 512], F32, tag="acc", name=f"acc{i}_{kt}_{ci}"
                    )
                    for dp in range(ND // 2):
                        nc.tensor.matmul(
                            acc[:, 0:qn_],
                            cT[:, 2 * dp : 2 * dp + 2, kt * P : (kt + 1) * P],
                            qT[:, 2 * dp : 2 * dp + 2, q0 : q0 + qn_],
                            start=(dp == 0),
                            stop=(dp == ND // 2 - 1),
                            perf_mode=mybir.MatmulPerfMode.DoubleRow,
                        )
                    nc.scalar.activation(
                        sg[:, kt, q0 : q0 + qn_], acc[:, 0:qn_],
                        AF.Sigmoid, bias=kb[:, kt : kt + 1],
                        scale=inv_c[:, kt : kt + 1],
                    )
            s_["sg"] = sg
            s_["qchunks"] = qchunks

        def emit_mm2(i):
            """den in row form (ones-column matmuls over sigT), transposed to
            per-partition columns on the PE; then per q-block att + sigT
            transposes, evicted with w = qmask * min(1/den, 1)."""
            s_ = st[i]
            lb, ct, qm, sg = s_["lb"], s_["ct"], s_["qm"], s_["sg"]
            qchunks = s_["qchunks"]
            nb = lb * P

            # den rows: den[0, q] = sum_k sigT[k, q]
            den_sb = mpool.tile([1, S], BF, tag="den", name=f"den{i}")
            for ci, (q0, qn_) in enumerate(qchunks):
                dp = ps1.tile([P, 512], F32, tag="acc", name=f"dp{i}_{ci}")
                for kt in range(lb):
                    nc.tensor.matmul(
                        dp[0:1, 0:qn_], ones[:, 0:1], sg[:, kt, q0 : q0 + qn_],
                        start=(kt == 0), stop=(kt == lb - 1),
                    )
                nc.scalar.copy(den_sb[0:1, q0 : q0 + qn_], dp[0:1, 0:qn_])
            # transpose den to columns: dcol[p, qb] = den[qb*P + p]
            dcol = psd.tile([P, NT, 2], BF, tag="dcol", name=f"dcol{i}")
            for qb in range(lb):
                nc.tensor.transpose(
                    dcol[:, qb, 0:1],
                    den_sb[0:1, qb * P : (qb + 1) * P],
                    ones[0:1, 0:1],
                )
            # w_all = qmask * min(1/den, 1)  (den > 0 always)
            winv = wpool.tile([P, NT], F32, tag="winv", name=f"wi{i}")
            w_all = wpool.tile([P, NT], F32, tag="w", name=f"w{i}")
            nc.vector.reciprocal(winv[:, 0:lb], dcol[:, 0:lb, 0])
            nc.vector.scalar_tensor_tensor(
                w_all[:, 0:lb], winv[:, 0:lb], 1.0, qm[:, 0:lb],
                op0=ALU.min, op1=ALU.mult,
            )

            ao_all = opool.tile([P, NT, D], BF, tag="ao", name=f"ao{i}")
            so_all = opool.tile([P, NT, S], BF, tag="so", name=f"so{i}")
            for qb in range(lb):
                att = ps2.tile([P, 512], F32, tag="att", name=f"att{i}_{qb}")
                pt = pst.tile([P, NT * P], BF, tag="pt", name=f"pt{i}_{qb}")
                for kt in range(lb):
                    sgblk = sg[:, kt, qb * P : (qb + 1) * P]
                    nc.tensor.matmul(
                        att[:], sgblk, ct[:, kt],
                        start=(kt == 0), stop=(kt == lb - 1),
                    )
                    nc.tensor.transpose(
                        pt[:, kt * P : (kt + 1) * P], sgblk, ident[:]
                    )
                w = w_all[:, qb : qb + 1]
                if qb % 2 == 0:
                    nc.vector.tensor_scalar_mul(ao_all[:, qb], att[:], w)
                else:
                    nc.scalar.activation(ao_all[:, qb], att[:], AF.Copy, scale=w)
                if qb % 2 == 0:
                    nc.scalar.activation(
                        so_all[:, qb, 0:nb], pt[:, 0:nb], AF.Copy, scale=w
                    )
                else:
                    nc.vector.tensor_scalar_mul(
                        so_all[:, qb, 0:nb], pt[:, 0:nb], w
                    )
            nc.gpsimd.dma_start(
                atts[i].rearrange("(t p) d -> p t d", p=P), ao_all[:, 0:lb]
            )
            screar = scs[i].rearrange("(t p) k -> p t k", p=P)
            if lb >= 4:
                h = lb // 2
                nc.gpsimd.dma_start(screar[:, 0:h], so_all[:, 0:h, 0:nb])
                nc.gpsimd.dma_start(screar[:, h:lb], so_all[:, h:lb, 0:nb])
            else:
                nc.gpsimd.dma_start(screar[:, 0:lb], so_all[:, 0:lb, 0:nb])

        front_pre(0)
        front_T(0)
        for i in range(NSLOTS):
            emit_mm1(i)
            if i + 1 < NSLOTS:
                front_pre(i + 1)
            emit_mm2(i)
            if i + 1 < NSLOTS:
                front_T(i + 1)


_NC_CACHE = {}


def _get_nc(slot_lbs):
    key = tuple(slot_lbs)
    if key not in _NC_CACHE:
        _NC_CACHE[key] = build_kernel(key)
    return _NC_CACHE[key]


def _plan(length):
    """Sort batches desc by length, deal rank r -> (slot r//8, core r%8)."""
    order = np.argsort(-length, kind="stable")
    slot_lbs = []
    for i in range(NSLOTS):
        lmax = int(length[order[i * NCORES]])
        for r in range(i * NCORES, (i + 1) * NCORES):
            lmax = max(lmax, int(length[order[r]]))
        slot_lbs.append(max(1, (lmax + P - 1) // P))
    return order, tuple(slot_lbs)


def kernel(context, query, length):
    context = np.asarray(context, dtype=np.float32)
    query = np.asarray(query, dtype=np.float32)
    length = np.asarray(length).astype(np.int64)

    order, slot_lbs = _plan(length)

    q_bf = query.astype(BF16)
    c_bf = context.astype(BF16)
    iot = np.arange(S)
    keymask = iot[None, :] < length[:, None]                      # [B, S]
    kbH = np.where(keymask, np.float32(0.0), NEG).astype(np.float32)
    kbH = np.ascontiguousarray(kbH.reshape(B, NT, P).transpose(0, 2, 1))
    qmH = keymask.astype(np.float32)
    qmH = np.ascontiguousarray(qmH.reshape(B, NT, P).transpose(0, 2, 1))

    in_maps = []
    for c in range(NCORES):
        m = {
            "ident": np.eye(P, dtype=np.float32).astype(BF16),
            "ones": np.ones((P, 2), dtype=np.float32).astype(BF16),
        }
        for i, lb in enumerate(slot_lbs):
            b = int(order[i * NCORES + c])
            nb = lb * P
            m[f"q{i}"] = np.ascontiguousarray(q_bf[b])
            m[f"c{i}"] = np.ascontiguousarray(c_bf[b, :nb])
            m[f"kb{i}"] = np.ascontiguousarray(kbH[b, :, :lb])
            m[f"qm{i}"] = np.ascontiguousarray(qmH[b, :, :lb])
        in_maps.append(m)

    nc = _get_nc(slot_lbs)
    res = run_bass_kernel_spmd(nc, in_maps, list(range(NCORES)))
    _NC_CACHE["last_result"] = res

    out = np.zeros((B, S, 2 * D), dtype=np.float32)
    scores = np.zeros((B, S, S), dtype=np.float32)
    for i, lb in enumerate(slot_lbs):
        nb = lb * P
        for c in range(NCORES):
            b = int(order[i * NCORES + c])
            r = res.results[c]
            out[b, :, 0:D] = r[f"qn{i}"].astype(np.float32)
            out[b, 0:nb, D : 2 * D] = r[f"att{i}"].astype(np.float32)
            scores[b, 0:nb, 0:nb] = r[f"sc{i}"].astype(np.float32)
    return out, scores
